# revision 1
# baseline (speedup 1.0000x reference)
"""BitLinear (RMSNorm + per-tensor 8-bit act quant + ternary weight quant + matmul)
as a distributed Bass/Tile kernel on 8 TRN2 NeuronCores.

Sharding: data-parallel over tokens (B*S = 32768 -> 4096 tokens/core).
Every core loads the full (host-pre-transposed) weight and computes
w_scale redundantly; the only collective is an AllGather of the 8
per-core |xn| maxima (per-tensor activation scale needs a global max).

Key numerical trick: after quantization x_q in [-127,127] (integers) and
w_q in {-1,0,1}, so a bf16 matmul with fp32 PSUM accumulation is EXACT
(all intermediate integers < 2^24).  Round-to-nearest-even is implemented
with the fp32 magic-constant trick (add/sub 1.5*2^23), matching jnp.round.
"""

import numpy as np

# ---- problem constants (hardcoded per contract) ----
B, S, DIN, DOUT = 4, 8192, 1024, 1024
N_CORES = 8
TOK = B * S                    # 32768 tokens
TOK_C = TOK // N_CORES         # 4096 tokens per core
TPD = 256                      # tokens per DMA tile (2 x 128)
ND = TOK_C // TPD              # 16 DMA tiles per core
SUB = TPD // 128               # 2 sub-tiles of 128 tokens per DMA tile
NT = TOK_C // 128              # 32 stat columns (one per 128-token sub-tile)
KT = DIN // 128                # 8 contraction tiles
NH = DOUT // 512               # 2 psum halves of the output row
EPS = 1e-6
QP = 127.0
MAGIC = 12582912.0             # 1.5 * 2**23: fp32 RNE round-to-int trick

_CACHE = {}


def _build(apply_nw: bool):
    import concourse.bass as bass
    import concourse.bacc as bacc
    import concourse.mybir as mybir
    from concourse import tile, masks

    f32 = mybir.dt.float32
    bf16 = mybir.dt.bfloat16
    fp16 = mybir.dt.float16
    AF = mybir.ActivationFunctionType
    OP = mybir.AluOpType
    AX = mybir.AxisListType

    nc = bacc.Bacc("TRN2", target_bir_lowering=False, debug=False,
                   num_devices=N_CORES)

    x_d = nc.dram_tensor("x", [TOK_C, DIN], f32, kind="ExternalInput")
    wt_d = nc.dram_tensor("wt", [DIN, DOUT], f32, kind="ExternalInput")
    if apply_nw:
        nw_d = nc.dram_tensor("nw", [1, DIN], f32, kind="ExternalInput")
    out_d = nc.dram_tensor("out", [TOK_C, DOUT], f32, kind="ExternalOutput")

    with tile.TileContext(nc) as tc:
        with (
            tc.tile_pool(name="const", bufs=1) as const_pool,
            tc.tile_pool(name="stats", bufs=1) as stats,
            tc.tile_pool(name="xs", bufs=3) as x_pool,
            tc.tile_pool(name="xnT", bufs=NT) as xnT_pool,
            tc.tile_pool(name="wqs", bufs=KT) as wq_pool,
            tc.tile_pool(name="dram", bufs=1, space="DRAM") as dram_pool,
            tc.tile_pool(name="psS", bufs=1, space="PSUM") as psS,
        ):
            # ---------- constants ----------
            ident_bf = const_pool.tile([128, 128], fp16, tag="ident_bf")
            masks.make_identity(nc, ident_bf[:, :])
            ident_f32 = const_pool.tile([128, 128], f32, tag="ident_f32")
            masks.make_identity(nc, ident_f32[:, :])
            ones_row = const_pool.tile([1, 128], f32, tag="ones_row")
            nc.gpsimd.memset(ones_row[:, :], 1.0)

            # stat tiles
            sumsq = stats.tile([128, NT], f32, tag="sumsq")
            amax = stats.tile([128, NT], f32, tag="amax")
            rms = stats.tile([128, NT], f32, tag="rms")
            wsum = stats.tile([128, KT], f32, tag="wsum")

            def bcast_scalar(src, tag):
                """[1,1] fp32 -> [128,1] via ones-matmul (broadcast along partitions)."""
                pb = psS.tile([128, 1], f32, tag="pb", name="pb_" + tag)
                nc.tensor.matmul(pb[:, :], lhsT=ones_row[:, :], rhs=src,
                                 start=True, stop=True)
                dst = stats.tile([128, 1], f32, tag=tag, name=tag)
                nc.vector.tensor_copy(dst[:, :], pb[:, :])
                return dst

            def part_reduce(vec128, op, tag):
                """[128,1] fp32 -> [1,1] via PE transpose + DVE reduce."""
                pt = psS.tile([1, 128], f32, tag="pt", name="pt_" + tag)
                nc.tensor.transpose(pt[:, :], vec128, ident_f32[:, :])
                sb = stats.tile([1, 128], f32, tag=tag + "_row", name=tag + "_row")
                nc.vector.tensor_copy(sb[:, :], pt[:, :])
                r = stats.tile([1, 1], f32, tag=tag, name=tag)
                nc.vector.tensor_reduce(out=r[:, :], in_=sb[:, :], axis=AX.X, op=op)
                return r

            x_tiles = []
            wq_tiles = []

            with (
                tc.tile_pool(name="wts", bufs=KT) as wt_pool,
                tc.tile_pool(name="fscr", bufs=2) as fscr_pool,
            ):
                # ---------- norm_weight broadcast (general path only) ----------
                if apply_nw:
                    nw_sb = stats.tile([1, DIN], f32, tag="nw_sb")
                    nc.sync.dma_start(out=nw_sb[:, :], in_=nw_d[:, :])
                    nwb = const_pool.tile([128, DIN], f32, tag="nwb")
                    for h in range(2):
                        nwp = psS.tile([128, 512], f32, tag="nwb_ps",
                                       name=f"nwb_ps{h}")
                        nc.tensor.matmul(nwp[:, :], lhsT=ones_row[:, :],
                                         rhs=nw_sb[:, h * 512:(h + 1) * 512],
                                         start=True, stop=True)
                        nc.vector.tensor_copy(nwb[:, h * 512:(h + 1) * 512],
                                              nwp[:, :])

                # ---------- x: load + stats; apply rms + transpose early so
                # phase B only needs the per-tensor quant scale (k-major OK) --
                with tc.tile_pool(name="psA", bufs=2, space="PSUM") as psA:
                    for d in range(ND):
                        xt = x_pool.tile([128, SUB, DIN], f32, tag="xt")
                        nc.sync.dma_start(
                            out=xt[:, :, :],
                            in_=x_d[d * TPD:(d + 1) * TPD, :].rearrange(
                                "(c p) k -> p c k", p=128))
                        if apply_nw:
                            xh = x_pool.tile([128, SUB, DIN], f32, tag="xh")
                            for c in range(SUB):
                                nc.vector.tensor_tensor(out=xh[:, c, :],
                                                        in0=xt[:, c, :],
                                                        in1=nwb[:, :],
                                                        op=OP.mult)
                        else:
                            xh = xt
                        for c in range(SUB):
                            scr = fscr_pool.tile([128, DIN], f32, tag="fscr")
                            nc.scalar.activation(
                                out=scr[:, :], in_=xt[:, c, :], func=AF.Square,
                                accum_out=sumsq[:, d * SUB + c:d * SUB + c + 1])
                        nc.vector.tensor_reduce(
                            out=amax[:, d * SUB:(d + 1) * SUB], in_=xh[:, :, :],
                            axis=AX.X, op=OP.max, apply_absolute_value=True)
                        # rms for these two sub-tiles
                        sl = slice(d * SUB, (d + 1) * SUB)
                        m2 = stats.tile([128, SUB], f32, tag="m2",
                                        name=f"m2_{d}")
                        nc.vector.tensor_scalar(out=m2[:, :],
                                                in0=sumsq[:, sl],
                                                scalar1=1.0 / DIN, scalar2=EPS,
                                                op0=OP.mult, op1=OP.add)
                        r2 = stats.tile([128, SUB], f32, tag="r2",
                                        name=f"r2_{d}")
                        nc.vector.reciprocal(r2[:, :], m2[:, :])
                        nc.scalar.activation(out=rms[:, sl], in_=r2[:, :],
                                             func=AF.Sqrt)
                        for c in range(SUB):
                            i = d * SUB + c
                            xn = x_pool.tile([128, DIN], fp16, tag="xn")
                            nc.scalar.activation(out=xn[:, :],
                                                 in_=xh[:, c, :],
                                                 func=AF.Copy,
                                                 scale=rms[:, i:i + 1])
                            pA = psA.tile([128, DIN], fp16, tag="pA")
                            for j in range(KT):
                                nc.tensor.transpose(
                                    pA[:, j * 128:(j + 1) * 128],
                                    xn[:, j * 128:(j + 1) * 128],
                                    ident_bf[:, :])
                            xnT = xnT_pool.tile([128, DIN], fp16, tag="xnT")
                            nc.vector.tensor_copy(xnT[:, :], pA[:, :])
                            x_tiles.append(xnT)

                # ---------- local |xn| max (rms already computed per tile) ----
                axn = stats.tile([128, NT], f32, tag="axn")
                nc.vector.tensor_tensor(out=axn[:, :], in0=amax[:, :],
                                        in1=rms[:, :], op=OP.mult)
                # reference clips xn to +-1e4 (nan_to_num) before the abs-max
                axn2 = stats.tile([128, NT], f32, tag="axn2")
                nc.vector.tensor_scalar(out=axn2[:, :], in0=axn[:, :],
                                        scalar1=1e4, scalar2=None, op0=OP.min)
                lmax = stats.tile([128, 1], f32, tag="lmax")
                nc.vector.tensor_reduce(out=lmax[:, :], in_=axn2[:, :],
                                        axis=AX.X, op=OP.max)
                gmax0 = part_reduce(lmax[:, :], OP.max, "gmax0")

                # ---------- collective: AllGather the 8 local maxima ----------
                cc_in = dram_pool.tile([1, 1], f32, tag="cc_in")
                cc_out = dram_pool.tile([1, N_CORES], f32, tag="cc_out")
                nc.sync.dma_start(out=cc_in[:, :], in_=gmax0[:, :])
                nc.gpsimd.collective_compute(
                    "AllGather", OP.bypass,
                    replica_groups=[list(range(N_CORES))],
                    ins=[cc_in[:, :].opt()],
                    outs=[cc_out[:, :].opt()],
                )
                # ---------- weight path (fills the collective-wait bubble) ----
                wt_tiles = []
                for j in range(KT):
                    wtt = wt_pool.tile([128, DOUT], f32, tag="wt")
                    nc.sync.dma_start(out=wtt[:, :],
                                      in_=wt_d[j * 128:(j + 1) * 128, :])
                    wt_tiles.append(wtt)
                    scr = fscr_pool.tile([128, DOUT], f32, tag="fscr")
                    nc.scalar.activation(out=scr[:, :], in_=wtt[:, :],
                                         func=AF.Abs,
                                         accum_out=wsum[:, j:j + 1])

                wred = stats.tile([128, 1], f32, tag="wred")
                nc.vector.tensor_reduce(out=wred[:, :], in_=wsum[:, :],
                                        axis=AX.X, op=OP.add)
                wtot = part_reduce(wred[:, :], OP.add, "wtot")
                wsc = stats.tile([1, 1], f32, tag="wsc")
                nc.vector.tensor_scalar(out=wsc[:, :], in0=wtot[:, :],
                                        scalar1=1.0 / (DIN * DOUT),
                                        scalar2=1e-4, op0=OP.mult, op1=OP.max)
                inv_ws = stats.tile([1, 1], f32, tag="inv_ws")
                nc.vector.reciprocal(inv_ws[:, :], wsc[:, :])
                inv_ws_b = bcast_scalar(inv_ws[:, :], "inv_ws_b")

                for j in range(KT):
                    # t1 = MAGIC + round(w/ws): the fp32 add itself rounds RNE
                    q1 = fscr_pool.tile([128, DOUT], f32, tag="fscr")
                    nc.scalar.activation(out=q1[:, :], in_=wt_tiles[j][:, :],
                                         func=AF.Copy,
                                         scale=inv_ws_b[:, 0:1], bias=MAGIC)
                    q2 = fscr_pool.tile([128, DOUT], f32, tag="fscr")
                    nc.vector.tensor_scalar(out=q2[:, :], in0=q1[:, :],
                                            scalar1=MAGIC, scalar2=1.0,
                                            op0=OP.subtract, op1=OP.min)
                    wq = wq_pool.tile([128, DOUT], fp16, tag="wq")
                    nc.vector.tensor_scalar(out=wq[:, :], in0=q2[:, :],
                                            scalar1=-1.0, scalar2=None,
                                            op0=OP.max)
                    wq_tiles.append(wq)

                cc_sb = stats.tile([1, N_CORES], f32, tag="cc_sb")
                nc.sync.dma_start(out=cc_sb[:, :], in_=cc_out[:, :])
                g1 = stats.tile([1, 1], f32, tag="g1")
                nc.vector.tensor_reduce(out=g1[:, :], in_=cc_sb[:, :],
                                        axis=AX.X, op=OP.max)
                a_scale = stats.tile([1, 1], f32, tag="a_scale")
                nc.vector.tensor_scalar(out=a_scale[:, :], in0=g1[:, :],
                                        scalar1=1e-5, scalar2=None, op0=OP.max)

                # per-tensor quant scale 127 / a_scale (rms already applied)
                inv_a = stats.tile([1, 1], f32, tag="inv_a")
                nc.vector.reciprocal(inv_a[:, :], a_scale[:, :])
                q127 = stats.tile([1, 1], f32, tag="q127")
                nc.vector.tensor_scalar(out=q127[:, :], in0=inv_a[:, :],
                                        scalar1=QP, scalar2=None, op0=OP.mult)
                qb = bcast_scalar(q127[:, :], "qb")

                # output scale c = a_scale * w_scale / 127
                c0 = stats.tile([1, 1], f32, tag="c0")
                nc.vector.tensor_tensor(out=c0[:, :], in0=a_scale[:, :],
                                        in1=wsc[:, :], op=OP.mult)
                c1 = stats.tile([1, 1], f32, tag="c1")
                nc.vector.tensor_scalar(out=c1[:, :], in0=c0[:, :],
                                        scalar1=1.0 / QP, scalar2=None,
                                        op0=OP.mult)
                cb = bcast_scalar(c1[:, :], "cb")

            # ---------- phase B: uniform-scale quantize in k-major + matmul --
            with (
                tc.tile_pool(name="aq", bufs=3) as aq_pool,
                tc.tile_pool(name="xqT", bufs=3) as xqT_pool,
                tc.tile_pool(name="outp", bufs=2) as out_pool,
                tc.tile_pool(name="psO", bufs=4, space="PSUM") as psO,
            ):
                for d in range(ND):
                    ot = out_pool.tile([128, SUB, DOUT], f32, tag="ot")
                    for c in range(SUB):
                        i = d * SUB + c
                        aq = aq_pool.tile([128, DIN], f32, tag="aq")
                        nc.scalar.activation(out=aq[:, :],
                                             in_=x_tiles[i][:, :],
                                             func=AF.Copy,
                                             scale=qb[:, 0:1], bias=MAGIC)
                        xqT = xqT_pool.tile([128, DIN], fp16, tag="xqT")
                        nc.vector.tensor_scalar(out=xqT[:, :], in0=aq[:, :],
                                                scalar1=MAGIC, scalar2=None,
                                                op0=OP.subtract)

                        po = [psO.tile([128, 512], f32, tag="po",
                                       name=f"po{h}") for h in range(NH)]
                        for j in range(KT):
                            for h in range(NH):
                                nc.tensor.matmul(
                                    po[h][:, :],
                                    lhsT=xqT[:, j * 128:(j + 1) * 128],
                                    rhs=wq_tiles[j][:, h * 512:(h + 1) * 512],
                                    start=(j == 0), stop=(j == KT - 1))
                        for h in range(NH):
                            nc.vector.tensor_scalar(
                                out=ot[:, c, h * 512:(h + 1) * 512],
                                in0=po[h][:, :], scalar1=cb[:, 0:1],
                                scalar2=None, op0=OP.mult)
                    nc.sync.dma_start(
                        out=out_d[d * TPD:(d + 1) * TPD, :].rearrange(
                            "(c p) n -> p c n", p=128),
                        in_=ot[:, :, :])

    nc.compile()
    return nc


def _get_nc(apply_nw: bool):
    key = ("nc", apply_nw)
    if key not in _CACHE:
        _CACHE[key] = _build(apply_nw)
    return _CACHE[key]


def _run(x, weight, norm_weight, trace=False):
    from concourse import bass_utils

    x = np.ascontiguousarray(np.asarray(x, dtype=np.float32))
    weight = np.ascontiguousarray(np.asarray(weight, dtype=np.float32))
    norm_weight = np.asarray(norm_weight, dtype=np.float32)

    apply_nw = not bool(np.all(norm_weight == 1.0))
    nc = _get_nc(apply_nw)

    xf = x.reshape(TOK, DIN)
    wt = np.ascontiguousarray(weight.T)          # [DIN, DOUT]
    in_maps = []
    for c in range(N_CORES):
        m = {"x": np.ascontiguousarray(xf[c * TOK_C:(c + 1) * TOK_C]),
             "wt": wt}
        if apply_nw:
            m["nw"] = norm_weight.reshape(1, DIN)
        in_maps.append(m)

    res = bass_utils.run_bass_kernel_spmd(
        nc, in_maps, core_ids=list(range(N_CORES)), trace=trace)

    out = np.empty((TOK, DOUT), dtype=np.float32)
    for c in range(N_CORES):
        out[c * TOK_C:(c + 1) * TOK_C] = res.results[c]["out"]
    return out.reshape(B, S, DOUT), res


def kernel(x, weight, norm_weight):
    out, _ = _run(x, weight, norm_weight, trace=False)
    return out

